# revision 34
# baseline (speedup 1.0000x reference)
"""Census consistency layer (segment-sum + gather) on 8 Trainium2 cores.

Sharding: data-parallel over batch B=16 -> 8 cores x 2 batches each.
Per batch (1M pixels, 256 admin bins):
  S[a]   = sum of P_raw over pixels with admin_id == a   (masked, -1 invalid)
  R[a]   = census[a] / (S[a] + eps)
  out[n] = valid ? P_raw[n] * R[id[n]] : P_raw[n]
"""

import sys

sys.path.insert(0, "/opt/trn_rl_repo")

import numpy as np

import concourse.bacc as bacc
import concourse.tile as tile
from concourse import mybir
from concourse.bass_interp import get_hw_module
from concourse.bass_utils import run_bass_kernel_spmd

B, H, W, A = 16, 1024, 1024, 256
NCORES = 8
BPC = B // NCORES  # batches per core
P = 128
F = (H * W) // P  # free-dim elements per partition (8192)
EPS = 1e-6

_cache: dict = {}


def _build_body(nc, tc, p_ext, ids_ext, cen_ext, out_ext, r_scratch):
    import concourse.bass as bass
    from concourse.masks import make_identity

    f32 = mybir.dt.float32
    bf16 = mybir.dt.bfloat16
    WIN = 512
    with (
        tc.tile_pool(name="big", bufs=1) as big,
        tc.tile_pool(name="small", bufs=1) as small,
        tc.tile_pool(name="winp", bufs=3) as winp,
        tc.tile_pool(name="psum", bufs=2, space="PSUM") as psum,
        tc.tile_pool(name="pbig", bufs=1, space="PSUM") as pbig,
    ):
        ones = small.tile([P, 1], f32, tag="ones")
        nc.vector.memset(ones, 1.0)

        # iotaL[p] = p % 16 as f32 [128,1]
        iota_i = small.tile([P, 1], mybir.dt.int32, tag="iota_i")
        nc.gpsimd.iota(iota_i[:], pattern=[[0, 1]], base=0, channel_multiplier=1)
        nc.vector.tensor_scalar(
            out=iota_i[:],
            in0=iota_i[:],
            scalar1=15,
            scalar2=None,
            op0=mybir.AluOpType.bitwise_and,
        )
        iota_l = small.tile([P, 1], f32, tag="iota_l")
        nc.vector.tensor_copy(out=iota_l[:], in_=iota_i[:])

        # bd-structured PE constants: ones_bd[16q+l, q] = 1
        # (engine APs must start at aligned partitions, so build via
        #  full-partition compares against q = p >> 4)
        hiq_i = small.tile([P, 1], mybir.dt.int32, tag="hiq_i")
        nc.gpsimd.iota(hiq_i[:], pattern=[[0, 1]], base=0, channel_multiplier=1)
        nc.vector.tensor_scalar(
            out=hiq_i[:],
            in0=hiq_i[:],
            scalar1=4,
            scalar2=None,
            op0=mybir.AluOpType.logical_shift_right,
        )
        hiq_f = small.tile([P, 1], f32, tag="hiq_f")
        nc.vector.tensor_copy(out=hiq_f[:], in_=hiq_i[:])
        ones_bd = small.tile([P, 8], bf16, tag="ones_bd")
        for q in range(8):
            nc.vector.tensor_scalar(
                out=ones_bd[:, q : q + 1],
                in0=hiq_f[:],
                scalar1=float(q),
                scalar2=None,
                op0=mybir.AluOpType.is_equal,
            )
        rm_bd = small.tile([P, P], bf16, tag="rm_bd")
        nc.vector.memset(rm_bd[:], 0.0)

        iota16_i = small.tile([P, 16], mybir.dt.int32, tag="iota16_i")
        nc.gpsimd.iota(iota16_i[:], pattern=[[1, 16]], base=0, channel_multiplier=0)
        iota16_bf = small.tile([P, 16], bf16, tag="iota16_bf")
        nc.vector.tensor_copy(out=iota16_bf[:], in_=iota16_i[:])
        ident_bf = small.tile([P, P], bf16, tag="ident")
        make_identity(nc, ident_bf[:])

        # st_si[k, 16q+l] = ((k mod 32) == 8*si + q): PE broadcast stationaries,
        # replicated every 32 partitions so lhsT can share rhs's base partition.
        iota_k = small.tile([P, 1], mybir.dt.int32, tag="iota_k")
        nc.gpsimd.iota(iota_k[:], pattern=[[0, 1]], base=0, channel_multiplier=1)
        iota_kf = small.tile([P, 1], f32, tag="iota_kf")
        nc.vector.tensor_copy(out=iota_kf[:], in_=iota_k[:])
        st_oct = []
        for o in range(16):
            st = small.tile([P, P], bf16, tag=f"st_oct{o}")
            for q in range(8):
                sl = st[:, 16 * q : 16 * q + 16]
                nc.vector.tensor_scalar(
                    out=sl,
                    in0=bass.AP(
                        tensor=iota_kf.tensor,
                        offset=iota_kf[:].offset,
                        ap=[iota_kf[:].ap[0], [0, 16]],
                    ),
                    scalar1=float(8 * o + q),
                    scalar2=None,
                    op0=mybir.AluOpType.is_equal,
                )
            st_oct.append(st)

        for b in range(BPC):
            # --- load ---
            p_t = big.tile([P, F], f32, tag="p")
            nc.sync.dma_start(out=p_t[:], in_=p_ext[b])
            ids_t = big.tile([P, F], mybir.dt.int32, tag="ids")
            nc.sync.dma_start(out=ids_t[:], in_=ids_ext[b])

            # contrib = (ids >= 0) * p, in bf16 (S only needs ~1e-3 rel)
            contrib_bf = big.tile([P, F], bf16, tag="contrib_bf")
            nc.vector.scalar_tensor_tensor(
                out=contrib_bf[:],
                in0=ids_t[:],
                scalar=0,
                in1=p_t[:],
                op0=mybir.AluOpType.is_ge,
                op1=mybir.AluOpType.mult,
            )

            # hi/lo via int ops; invalid ids (-1) produce hi >= 16 so every
            # hi-mask is false -> g=0 there, and contrib=0 keeps the hist exact
            t_i = big.tile([P, F], mybir.dt.int32, tag="idf")
            nc.vector.tensor_scalar(
                out=t_i[:],
                in0=ids_t[:],
                scalar1=4,
                scalar2=None,
                op0=mybir.AluOpType.logical_shift_right,
            )
            hi_bf = big.tile([P, F], bf16, tag="hi_bf")
            nc.scalar.copy(out=hi_bf[:], in_=t_i[:])
            t_i2 = big.tile([P, F], mybir.dt.int32, tag="idf")
            nc.vector.tensor_scalar(
                out=t_i2[:],
                in0=ids_t[:],
                scalar1=15,
                scalar2=None,
                op0=mybir.AluOpType.bitwise_and,
            )
            lo_bf = big.tile([P, F], bf16, tag="lo_bf")
            nc.scalar.copy(out=lo_bf[:], in_=t_i2[:])

            # --- histogram via PE diag-block matmuls ---
            # SP[16j+l, 16j+h] accumulates sum over pixels of V*U per 8-chunk set
            sp = pbig.tile([P, P], f32, tag="sp")
            n_chunks = F // P  # 64 transpose chunks
            first = True
            for c in range(n_chunks):
                cs = slice(c * P, (c + 1) * P)
                tr_ps = pbig.tile([P, 3 * P], bf16, tag="tr")
                nc.tensor.transpose(
                    out=tr_ps[:, 0:P], in_=hi_bf[:, cs], identity=ident_bf[:]
                )
                nc.tensor.transpose(
                    out=tr_ps[:, P : 2 * P], in_=lo_bf[:, cs], identity=ident_bf[:]
                )
                nc.tensor.transpose(
                    out=tr_ps[:, 2 * P : 3 * P],
                    in_=contrib_bf[:, cs],
                    identity=ident_bf[:],
                )
                # ct must land in SBUF for the gpsimd multiply (no PSUM port)
                ctt = winp.tile([P, P], bf16, tag="trio")
                nc.scalar.copy(out=ctt[:], in_=tr_ps[:, 2 * P : 3 * P])

                # U[p, j, h] = (hi_t[p, j] == h); pV[p, j, l] = (lo_t==l)*ct
                # read hi_t/lo_t directly from PSUM (PSUM-source DVE ops run
                # at spec; SBUF-source pay the 2.3x cayman penalty)
                hit = tr_ps[:, 0:P]
                lot = tr_ps[:, P : 2 * P]
                ctt = ctt[:]
                # one fused op: [hi_t | lo_t] vs iota16 -> [u | v] (halves op overhead)
                uv_c = winp.tile([P, 2, P, 16], bf16, tag="u")
                nc.vector.tensor_tensor(
                    out=uv_c[:],
                    in0=bass.AP(
                        tensor=hit.tensor,
                        offset=hit.offset,
                        ap=[hit.ap[0], [P, 2], hit.ap[1], [0, 16]],
                    ),
                    in1=bass.AP(
                        tensor=iota16_bf.tensor,
                        offset=iota16_bf[:].offset,
                        ap=[iota16_bf[:].ap[0], [0, 2], [0, P], iota16_bf[:].ap[1]],
                    ),
                    op=mybir.AluOpType.is_equal,
                )
                u_c = uv_c[:, 0]
                v_c = uv_c[:, 1]
                nc.gpsimd.tensor_tensor(
                    out=v_c,
                    in0=v_c,
                    in1=bass.AP(
                        tensor=ctt.tensor,
                        offset=ctt.offset,
                        ap=[ctt.ap[0], ctt.ap[1], [0, 16]],
                    ),
                    op=mybir.AluOpType.mult,
                )
                for s in range(P // 8):
                    last = c == n_chunks - 1 and s == P // 8 - 1
                    nc.tensor.matmul(
                        out=sp[:],
                        lhsT=v_c[:, 8 * s : 8 * s + 8, :],
                        rhs=u_c[:, 8 * s : 8 * s + 8, :],
                        start=first,
                        stop=last,
                    )
                    first = False

            # --- extract S16T[l, h] = sum_q SP[16q+l, 16q+h] ---
            sp_sb = small.tile([P, P], f32, tag="sp_sb")
            nc.scalar.copy(out=sp_sb[:], in_=sp[:])
            sblk = small.tile([16, 16, 8], f32, tag="sblk")
            for q in range(8):
                nc.sync.dma_start(
                    out=sblk[:, :, q : q + 1],
                    in_=sp_sb[16 * q : 16 * q + 16, 16 * q : 16 * q + 16],
                )
            s16t = small.tile([16, 16], f32, tag="s16t")
            nc.vector.tensor_reduce(
                out=s16t[:],
                in_=sblk[:],
                axis=mybir.AxisListType.X,
                op=mybir.AluOpType.add,
            )
            # R = census/(S+eps), transposed layout [l, h]
            nc.vector.tensor_scalar(
                out=s16t[:],
                in0=s16t[:],
                scalar1=EPS,
                scalar2=None,
                op0=mybir.AluOpType.add,
            )
            nc.vector.reciprocal(out=s16t[:], in_=s16t[:])
            cen_t = small.tile([16, 16], f32, tag="cen_t")
            cb = cen_ext[b]
            nc.sync.dma_start(
                out=cen_t[:],
                in_=bass.AP(tensor=cb.tensor, offset=cb.offset, ap=[[1, 16], [16, 16]]),
            )
            rm_t = small.tile([16, 16], f32, tag="rm_t")
            nc.vector.tensor_tensor(
                out=rm_t[:], in0=cen_t[:], in1=s16t[:], op=mybir.AluOpType.mult
            )
            rm_tbf = small.tile([16, 16], bf16, tag="rm_tbf")
            nc.vector.tensor_copy(out=rm_tbf[:], in_=rm_t[:])
            # RmBD[16q+l, 16q+h] = Rm[h, l]
            for q in range(8):
                nc.sync.dma_start(
                    out=rm_bd[16 * q : 16 * q + 16, 16 * q : 16 * q + 16],
                    in_=rm_tbf[:],
                )

            # --- octet gather: g[p, f] = R[idc[p, f]] ---
            gat = big.tile([P, F], bf16, tag="ids")  # reuse ids slot
            strip = small.tile([8, F], bf16, tag="strip")
            for o in range(16):
                for w in range(F // WIN):
                    ws = slice(w * WIN, (w + 1) * WIN)
                    bcp = psum.tile([P, 2, WIN], f32, tag="bc")
                    nc.tensor.matmul(
                        out=bcp[:, 0],
                        lhsT=st_oct[o][:],
                        rhs=lo_bf[:, ws],
                        start=True,
                        stop=True,
                    )
                    nc.tensor.matmul(
                        out=bcp[:, 1],
                        lhsT=st_oct[o][:],
                        rhs=hi_bf[:, ws],
                        start=True,
                        stop=True,
                    )
                    # one compare yields both the lo one-hot (W-matmul moving)
                    # and the hi one-hot (mux mask)
                    vtut = winp.tile([P, 2, WIN], bf16, tag="vt")
                    nc.vector.tensor_scalar(
                        out=vtut[:],
                        in0=bcp[:],
                        scalar1=iota_l[:],
                        scalar2=None,
                        op0=mybir.AluOpType.is_equal,
                    )
                    wt = pbig.tile([P, WIN], f32, tag="wt")
                    nc.tensor.matmul(
                        out=wt[:], lhsT=rm_bd[:], rhs=vtut[:, 0], start=True, stop=True
                    )
                    wt_sb = winp.tile([P, WIN], bf16, tag="hbw")
                    nc.scalar.copy(out=wt_sb[:], in_=wt[:])
                    prod = winp.tile([P, WIN], bf16, tag="prod")
                    nc.gpsimd.tensor_tensor(
                        out=prod[:],
                        in0=vtut[:, 1],
                        in1=wt_sb[:],
                        op=mybir.AluOpType.mult,
                    )
                    gps = pbig.tile([8, WIN], f32, tag="gps")
                    nc.tensor.matmul(
                        out=gps[:], lhsT=ones_bd[:], rhs=prod[:], start=True, stop=True
                    )
                    nc.scalar.copy(out=strip[:, ws], in_=gps[:])
                nc.sync.dma_start(out=gat[8 * o : 8 * o + 8, :], in_=strip[:])

            # --- out = contrib*g + (p - contrib), reusing p_t in place ---
            nc.gpsimd.tensor_tensor(
                out=gat[:], in0=contrib_bf[:], in1=gat[:], op=mybir.AluOpType.mult
            )
            nc.vector.tensor_tensor(
                out=p_t[:], in0=p_t[:], in1=contrib_bf[:], op=mybir.AluOpType.subtract
            )
            nc.vector.tensor_tensor(
                out=p_t[:], in0=p_t[:], in1=gat[:], op=mybir.AluOpType.add
            )
            nc.sync.dma_start(out=out_ext[b], in_=p_t[:])


def _build():
    nc = bacc.Bacc(
        "TRN2",
        target_bir_lowering=False,
        debug=False,
        enable_asserts=False,
        num_devices=NCORES,
    )
    f32 = mybir.dt.float32
    p_ext = nc.dram_tensor("p", [BPC, P, F], f32, kind="ExternalInput").ap()
    ids_ext = nc.dram_tensor(
        "ids", [BPC, P, F], mybir.dt.int32, kind="ExternalInput"
    ).ap()
    cen_ext = nc.dram_tensor("census", [BPC, A], f32, kind="ExternalInput").ap()
    out_ext = nc.dram_tensor("out", [BPC, P, F], f32, kind="ExternalOutput").ap()
    r_scratch = nc.dram_tensor("r_scratch", [1, A], f32).ap()
    with tile.TileContext(nc) as tc:
        _build_body(nc, tc, p_ext, ids_ext, cen_ext, out_ext, r_scratch)
    nc.compile()
    nc.m = get_hw_module(nc.m)
    return nc


def _run(P_raw, admin_ids, census_totals, trace=False):
    nc = _cache.get("nc")
    if nc is None:
        nc = _cache["nc"] = _build()
    in_maps = []
    for c in range(NCORES):
        sl = slice(c * BPC, (c + 1) * BPC)
        in_maps.append(
            {
                "p": np.ascontiguousarray(
                    np.asarray(P_raw[sl], dtype=np.float32).reshape(BPC, P, F)
                ),
                "ids": np.ascontiguousarray(
                    np.asarray(admin_ids[sl], dtype=np.int32).reshape(BPC, P, F)
                ),
                "census": np.ascontiguousarray(
                    np.asarray(census_totals[sl], dtype=np.float32)
                ),
            }
        )
    res = run_bass_kernel_spmd(nc, in_maps, list(range(NCORES)), trace=trace)
    out = np.concatenate(
        [res.results[c]["out"].reshape(BPC, 1, H, W) for c in range(NCORES)], axis=0
    ).astype(np.float32)
    return out, res


def kernel(P_raw, admin_ids, census_totals):
    out, _ = _run(P_raw, admin_ids, census_totals, trace=False)
    return out


# revision 35
# speedup vs baseline: 1.0891x; 1.0891x over previous
"""Census consistency layer (segment-sum + gather) on 8 Trainium2 cores.

Sharding: data-parallel over batch B=16 -> 8 cores x 2 batches each.
Per batch (1M pixels, 256 admin bins):
  S[a]   = sum of P_raw over pixels with admin_id == a   (masked, -1 invalid)
  R[a]   = census[a] / (S[a] + eps)
  out[n] = valid ? P_raw[n] * R[id[n]] : P_raw[n]
"""

import sys

sys.path.insert(0, "/opt/trn_rl_repo")

import numpy as np

import concourse.bacc as bacc
import concourse.tile as tile
from concourse import mybir
from concourse.bass_interp import get_hw_module
from concourse.bass_utils import run_bass_kernel_spmd

B, H, W, A = 16, 1024, 1024, 256
NCORES = 8
BPC = B // NCORES  # batches per core
P = 128
F = (H * W) // P  # free-dim elements per partition (8192)
EPS = 1e-6

_cache: dict = {}


def _build_body(nc, tc, p_ext, ids_ext, cen_ext, out_ext, r_scratch):
    import concourse.bass as bass
    from concourse.masks import make_identity

    f32 = mybir.dt.float32
    bf16 = mybir.dt.bfloat16
    WIN = 512
    with (
        tc.tile_pool(name="big", bufs=1) as big,
        tc.tile_pool(name="small", bufs=1) as small,
        tc.tile_pool(name="winp", bufs=3) as winp,
        tc.tile_pool(name="psum", bufs=2, space="PSUM") as psum,
        tc.tile_pool(name="pbig", bufs=1, space="PSUM") as pbig,
    ):
        ones = small.tile([P, 1], f32, tag="ones")
        nc.vector.memset(ones, 1.0)

        # iotaL[p] = p % 16 as f32 [128,1]
        iota_i = small.tile([P, 1], mybir.dt.int32, tag="iota_i")
        nc.gpsimd.iota(iota_i[:], pattern=[[0, 1]], base=0, channel_multiplier=1)
        nc.vector.tensor_scalar(
            out=iota_i[:],
            in0=iota_i[:],
            scalar1=15,
            scalar2=None,
            op0=mybir.AluOpType.bitwise_and,
        )
        iota_l = small.tile([P, 1], f32, tag="iota_l")
        nc.vector.tensor_copy(out=iota_l[:], in_=iota_i[:])

        # bd-structured PE constants: ones_bd[16q+l, q] = 1
        # (engine APs must start at aligned partitions, so build via
        #  full-partition compares against q = p >> 4)
        hiq_i = small.tile([P, 1], mybir.dt.int32, tag="hiq_i")
        nc.gpsimd.iota(hiq_i[:], pattern=[[0, 1]], base=0, channel_multiplier=1)
        nc.vector.tensor_scalar(
            out=hiq_i[:],
            in0=hiq_i[:],
            scalar1=4,
            scalar2=None,
            op0=mybir.AluOpType.logical_shift_right,
        )
        hiq_f = small.tile([P, 1], f32, tag="hiq_f")
        nc.vector.tensor_copy(out=hiq_f[:], in_=hiq_i[:])
        ones_bd = small.tile([P, 8], bf16, tag="ones_bd")
        for q in range(8):
            nc.vector.tensor_scalar(
                out=ones_bd[:, q : q + 1],
                in0=hiq_f[:],
                scalar1=float(q),
                scalar2=None,
                op0=mybir.AluOpType.is_equal,
            )
        rm_bd = small.tile([P, P], bf16, tag="rm_bd")
        nc.vector.memset(rm_bd[:], 0.0)

        iota16_i = small.tile([P, 16], mybir.dt.int32, tag="iota16_i")
        nc.gpsimd.iota(iota16_i[:], pattern=[[1, 16]], base=0, channel_multiplier=0)
        iota16_bf = small.tile([P, 16], bf16, tag="iota16_bf")
        nc.vector.tensor_copy(out=iota16_bf[:], in_=iota16_i[:])
        ident_bf = small.tile([P, P], bf16, tag="ident")
        make_identity(nc, ident_bf[:])

        # st_si[k, 16q+l] = ((k mod 32) == 8*si + q): PE broadcast stationaries,
        # replicated every 32 partitions so lhsT can share rhs's base partition.
        iota_k = small.tile([P, 1], mybir.dt.int32, tag="iota_k")
        nc.gpsimd.iota(iota_k[:], pattern=[[0, 1]], base=0, channel_multiplier=1)
        iota_kf = small.tile([P, 1], f32, tag="iota_kf")
        nc.vector.tensor_copy(out=iota_kf[:], in_=iota_k[:])
        st_oct = []
        for o in range(16):
            st = small.tile([P, P], bf16, tag=f"st_oct{o}")
            for q in range(8):
                sl = st[:, 16 * q : 16 * q + 16]
                nc.vector.tensor_scalar(
                    out=sl,
                    in0=bass.AP(
                        tensor=iota_kf.tensor,
                        offset=iota_kf[:].offset,
                        ap=[iota_kf[:].ap[0], [0, 16]],
                    ),
                    scalar1=float(8 * o + q),
                    scalar2=None,
                    op0=mybir.AluOpType.is_equal,
                )
            st_oct.append(st)

        for b in range(BPC):
            # --- load ---
            p_t = big.tile([P, F], f32, tag="p")
            nc.sync.dma_start(out=p_t[:], in_=p_ext[b])
            ids_t = big.tile([P, F], mybir.dt.int32, tag="ids")
            nc.sync.dma_start(out=ids_t[:], in_=ids_ext[b])

            # contrib = (ids >= 0) * p, in bf16 (S only needs ~1e-3 rel)
            contrib_bf = big.tile([P, F], bf16, tag="contrib_bf")
            nc.vector.scalar_tensor_tensor(
                out=contrib_bf[:],
                in0=ids_t[:],
                scalar=0,
                in1=p_t[:],
                op0=mybir.AluOpType.is_ge,
                op1=mybir.AluOpType.mult,
            )

            # hi/lo via int ops; invalid ids (-1) produce hi >= 16 so every
            # hi-mask is false -> g=0 there, and contrib=0 keeps the hist exact
            t_i = big.tile([P, F], mybir.dt.int32, tag="idf")
            nc.vector.tensor_scalar(
                out=t_i[:],
                in0=ids_t[:],
                scalar1=4,
                scalar2=None,
                op0=mybir.AluOpType.logical_shift_right,
            )
            hi_bf = big.tile([P, F], bf16, tag="hi_bf")
            nc.scalar.copy(out=hi_bf[:], in_=t_i[:])
            t_i2 = big.tile([P, F], mybir.dt.int32, tag="idf")
            nc.vector.tensor_scalar(
                out=t_i2[:],
                in0=ids_t[:],
                scalar1=15,
                scalar2=None,
                op0=mybir.AluOpType.bitwise_and,
            )
            lo_bf = big.tile([P, F], bf16, tag="lo_bf")
            nc.scalar.copy(out=lo_bf[:], in_=t_i2[:])

            # --- histogram via PE diag-block matmuls ---
            # SP[16j+l, 16j+h] accumulates sum over pixels of V*U per 8-chunk set
            sp = pbig.tile([P, P], f32, tag="sp")
            n_chunks = F // P  # 64 transpose chunks
            first = True
            for c in range(n_chunks):
                cs = slice(c * P, (c + 1) * P)
                tr_ps = psum.tile([P, 3 * P], bf16, tag="trwt")
                nc.tensor.transpose(
                    out=tr_ps[:, 0:P], in_=hi_bf[:, cs], identity=ident_bf[:]
                )
                nc.tensor.transpose(
                    out=tr_ps[:, P : 2 * P], in_=lo_bf[:, cs], identity=ident_bf[:]
                )
                nc.tensor.transpose(
                    out=tr_ps[:, 2 * P : 3 * P],
                    in_=contrib_bf[:, cs],
                    identity=ident_bf[:],
                )
                # ct must land in SBUF for the gpsimd multiply (no PSUM port)
                ctt = winp.tile([P, P], bf16, tag="trio")
                nc.scalar.copy(out=ctt[:], in_=tr_ps[:, 2 * P : 3 * P])

                # U[p, j, h] = (hi_t[p, j] == h); pV[p, j, l] = (lo_t==l)*ct
                # read hi_t/lo_t directly from PSUM (PSUM-source DVE ops run
                # at spec; SBUF-source pay the 2.3x cayman penalty)
                hit = tr_ps[:, 0:P]
                lot = tr_ps[:, P : 2 * P]
                ctt = ctt[:]
                # one fused op: [hi_t | lo_t] vs iota16 -> [u | v] (halves op overhead)
                uv_c = winp.tile([P, 2, P, 16], bf16, tag="u")
                nc.vector.tensor_tensor(
                    out=uv_c[:],
                    in0=bass.AP(
                        tensor=hit.tensor,
                        offset=hit.offset,
                        ap=[hit.ap[0], [P, 2], hit.ap[1], [0, 16]],
                    ),
                    in1=bass.AP(
                        tensor=iota16_bf.tensor,
                        offset=iota16_bf[:].offset,
                        ap=[iota16_bf[:].ap[0], [0, 2], [0, P], iota16_bf[:].ap[1]],
                    ),
                    op=mybir.AluOpType.is_equal,
                )
                u_c = uv_c[:, 0]
                v_c = uv_c[:, 1]
                nc.gpsimd.tensor_tensor(
                    out=v_c,
                    in0=v_c,
                    in1=bass.AP(
                        tensor=ctt.tensor,
                        offset=ctt.offset,
                        ap=[ctt.ap[0], ctt.ap[1], [0, 16]],
                    ),
                    op=mybir.AluOpType.mult,
                )
                for s in range(P // 8):
                    last = c == n_chunks - 1 and s == P // 8 - 1
                    nc.tensor.matmul(
                        out=sp[:],
                        lhsT=v_c[:, 8 * s : 8 * s + 8, :],
                        rhs=u_c[:, 8 * s : 8 * s + 8, :],
                        start=first,
                        stop=last,
                    )
                    first = False

            # --- extract S16T[l, h] = sum_q SP[16q+l, 16q+h] ---
            sp_sb = small.tile([P, P], f32, tag="sp_sb")
            nc.scalar.copy(out=sp_sb[:], in_=sp[:])
            sblk = small.tile([16, 16, 8], f32, tag="sblk")
            for q in range(8):
                nc.sync.dma_start(
                    out=sblk[:, :, q : q + 1],
                    in_=sp_sb[16 * q : 16 * q + 16, 16 * q : 16 * q + 16],
                )
            s16t = small.tile([16, 16], f32, tag="s16t")
            nc.vector.tensor_reduce(
                out=s16t[:],
                in_=sblk[:],
                axis=mybir.AxisListType.X,
                op=mybir.AluOpType.add,
            )
            # R = census/(S+eps), transposed layout [l, h]
            nc.vector.tensor_scalar(
                out=s16t[:],
                in0=s16t[:],
                scalar1=EPS,
                scalar2=None,
                op0=mybir.AluOpType.add,
            )
            nc.vector.reciprocal(out=s16t[:], in_=s16t[:])
            cen_t = small.tile([16, 16], f32, tag="cen_t")
            cb = cen_ext[b]
            nc.sync.dma_start(
                out=cen_t[:],
                in_=bass.AP(tensor=cb.tensor, offset=cb.offset, ap=[[1, 16], [16, 16]]),
            )
            rm_t = small.tile([16, 16], f32, tag="rm_t")
            nc.vector.tensor_tensor(
                out=rm_t[:], in0=cen_t[:], in1=s16t[:], op=mybir.AluOpType.mult
            )
            rm_tbf = small.tile([16, 16], bf16, tag="rm_tbf")
            nc.vector.tensor_copy(out=rm_tbf[:], in_=rm_t[:])
            # RmBD[16q+l, 16q+h] = Rm[h, l]
            for q in range(8):
                nc.sync.dma_start(
                    out=rm_bd[16 * q : 16 * q + 16, 16 * q : 16 * q + 16],
                    in_=rm_tbf[:],
                )

            # --- octet gather: g[p, f] = R[idc[p, f]] ---
            gat = big.tile([P, F], bf16, tag="ids")  # reuse ids slot
            strip = small.tile([8, F], bf16, tag="strip")
            for o in range(16):
                for w in range(F // WIN):
                    ws = slice(w * WIN, (w + 1) * WIN)
                    bcp = psum.tile([P, 2, WIN], f32, tag="bc")
                    nc.tensor.matmul(
                        out=bcp[:, 0],
                        lhsT=st_oct[o][:],
                        rhs=lo_bf[:, ws],
                        start=True,
                        stop=True,
                    )
                    nc.tensor.matmul(
                        out=bcp[:, 1],
                        lhsT=st_oct[o][:],
                        rhs=hi_bf[:, ws],
                        start=True,
                        stop=True,
                    )
                    # one compare yields both the lo one-hot (W-matmul moving)
                    # and the hi one-hot (mux mask)
                    vtut = winp.tile([P, 2, WIN], bf16, tag="vt")
                    nc.vector.tensor_scalar(
                        out=vtut[:],
                        in0=bcp[:],
                        scalar1=iota_l[:],
                        scalar2=None,
                        op0=mybir.AluOpType.is_equal,
                    )
                    wt = psum.tile([P, WIN], f32, tag="trwt")
                    nc.tensor.matmul(
                        out=wt[:], lhsT=rm_bd[:], rhs=vtut[:, 0], start=True, stop=True
                    )
                    wt_sb = winp.tile([P, WIN], bf16, tag="hbw")
                    nc.scalar.copy(out=wt_sb[:], in_=wt[:])
                    prod = winp.tile([P, WIN], bf16, tag="prod")
                    nc.gpsimd.tensor_tensor(
                        out=prod[:],
                        in0=vtut[:, 1],
                        in1=wt_sb[:],
                        op=mybir.AluOpType.mult,
                    )
                    gps = pbig.tile([8, WIN], f32, tag="gps")
                    nc.tensor.matmul(
                        out=gps[:], lhsT=ones_bd[:], rhs=prod[:], start=True, stop=True
                    )
                    nc.scalar.copy(out=strip[:, ws], in_=gps[:])
                nc.sync.dma_start(out=gat[8 * o : 8 * o + 8, :], in_=strip[:])

            # --- out = contrib*g + (p - contrib), reusing p_t in place ---
            nc.gpsimd.tensor_tensor(
                out=gat[:], in0=contrib_bf[:], in1=gat[:], op=mybir.AluOpType.mult
            )
            nc.vector.tensor_tensor(
                out=p_t[:], in0=p_t[:], in1=contrib_bf[:], op=mybir.AluOpType.subtract
            )
            nc.vector.tensor_tensor(
                out=p_t[:], in0=p_t[:], in1=gat[:], op=mybir.AluOpType.add
            )
            nc.sync.dma_start(out=out_ext[b], in_=p_t[:])


def _build():
    nc = bacc.Bacc(
        "TRN2",
        target_bir_lowering=False,
        debug=False,
        enable_asserts=False,
        num_devices=NCORES,
    )
    f32 = mybir.dt.float32
    p_ext = nc.dram_tensor("p", [BPC, P, F], f32, kind="ExternalInput").ap()
    ids_ext = nc.dram_tensor(
        "ids", [BPC, P, F], mybir.dt.int32, kind="ExternalInput"
    ).ap()
    cen_ext = nc.dram_tensor("census", [BPC, A], f32, kind="ExternalInput").ap()
    out_ext = nc.dram_tensor("out", [BPC, P, F], f32, kind="ExternalOutput").ap()
    r_scratch = nc.dram_tensor("r_scratch", [1, A], f32).ap()
    with tile.TileContext(nc) as tc:
        _build_body(nc, tc, p_ext, ids_ext, cen_ext, out_ext, r_scratch)
    nc.compile()
    nc.m = get_hw_module(nc.m)
    return nc


def _run(P_raw, admin_ids, census_totals, trace=False):
    nc = _cache.get("nc")
    if nc is None:
        nc = _cache["nc"] = _build()
    in_maps = []
    for c in range(NCORES):
        sl = slice(c * BPC, (c + 1) * BPC)
        in_maps.append(
            {
                "p": np.ascontiguousarray(
                    np.asarray(P_raw[sl], dtype=np.float32).reshape(BPC, P, F)
                ),
                "ids": np.ascontiguousarray(
                    np.asarray(admin_ids[sl], dtype=np.int32).reshape(BPC, P, F)
                ),
                "census": np.ascontiguousarray(
                    np.asarray(census_totals[sl], dtype=np.float32)
                ),
            }
        )
    res = run_bass_kernel_spmd(nc, in_maps, list(range(NCORES)), trace=trace)
    out = np.concatenate(
        [res.results[c]["out"].reshape(BPC, 1, H, W) for c in range(NCORES)], axis=0
    ).astype(np.float32)
    return out, res


def kernel(P_raw, admin_ids, census_totals):
    out, _ = _run(P_raw, admin_ids, census_totals, trace=False)
    return out


# revision 36
# speedup vs baseline: 1.1464x; 1.0526x over previous
"""Census consistency layer (segment-sum + gather) on 8 Trainium2 cores.

Sharding: data-parallel over batch B=16 -> 8 cores x 2 batches each.
Per batch (1M pixels, 256 admin bins):
  S[a]   = sum of P_raw over pixels with admin_id == a   (masked, -1 invalid)
  R[a]   = census[a] / (S[a] + eps)
  out[n] = valid ? P_raw[n] * R[id[n]] : P_raw[n]
"""

import sys

sys.path.insert(0, "/opt/trn_rl_repo")

import numpy as np

import concourse.bacc as bacc
import concourse.tile as tile
from concourse import mybir
from concourse.bass_interp import get_hw_module
from concourse.bass_utils import run_bass_kernel_spmd

B, H, W, A = 16, 1024, 1024, 256
NCORES = 8
BPC = B // NCORES  # batches per core
P = 128
F = (H * W) // P  # free-dim elements per partition (8192)
EPS = 1e-6

_cache: dict = {}


def _build_body(nc, tc, p_ext, ids_ext, cen_ext, out_ext, r_scratch):
    import concourse.bass as bass
    from concourse.masks import make_identity

    f32 = mybir.dt.float32
    bf16 = mybir.dt.bfloat16
    WIN = 512
    with (
        tc.tile_pool(name="big", bufs=1) as big,
        tc.tile_pool(name="small", bufs=1) as small,
        tc.tile_pool(name="winp", bufs=2) as winp,
        tc.tile_pool(name="winsm", bufs=6) as winsm,
        tc.tile_pool(name="psum", bufs=2, space="PSUM") as psum,
        tc.tile_pool(name="pbig", bufs=1, space="PSUM") as pbig,
    ):
        ones = small.tile([P, 1], f32, tag="ones")
        nc.vector.memset(ones, 1.0)

        # iotaL[p] = p % 16 as f32 [128,1]
        iota_i = small.tile([P, 1], mybir.dt.int32, tag="iota_i")
        nc.gpsimd.iota(iota_i[:], pattern=[[0, 1]], base=0, channel_multiplier=1)
        nc.vector.tensor_scalar(
            out=iota_i[:],
            in0=iota_i[:],
            scalar1=15,
            scalar2=None,
            op0=mybir.AluOpType.bitwise_and,
        )
        iota_l = small.tile([P, 1], f32, tag="iota_l")
        nc.vector.tensor_copy(out=iota_l[:], in_=iota_i[:])

        # bd-structured PE constants: ones_bd[16q+l, q] = 1
        # (engine APs must start at aligned partitions, so build via
        #  full-partition compares against q = p >> 4)
        hiq_i = small.tile([P, 1], mybir.dt.int32, tag="hiq_i")
        nc.gpsimd.iota(hiq_i[:], pattern=[[0, 1]], base=0, channel_multiplier=1)
        nc.vector.tensor_scalar(
            out=hiq_i[:],
            in0=hiq_i[:],
            scalar1=4,
            scalar2=None,
            op0=mybir.AluOpType.logical_shift_right,
        )
        hiq_f = small.tile([P, 1], f32, tag="hiq_f")
        nc.vector.tensor_copy(out=hiq_f[:], in_=hiq_i[:])
        ones_bd = small.tile([P, 8], bf16, tag="ones_bd")
        for q in range(8):
            nc.vector.tensor_scalar(
                out=ones_bd[:, q : q + 1],
                in0=hiq_f[:],
                scalar1=float(q),
                scalar2=None,
                op0=mybir.AluOpType.is_equal,
            )
        rm_bd = small.tile([P, P], bf16, tag="rm_bd")
        nc.vector.memset(rm_bd[:], 0.0)

        iota16_i = small.tile([P, 16], mybir.dt.int32, tag="iota16_i")
        nc.gpsimd.iota(iota16_i[:], pattern=[[1, 16]], base=0, channel_multiplier=0)
        iota16_bf = small.tile([P, 16], bf16, tag="iota16_bf")
        nc.vector.tensor_copy(out=iota16_bf[:], in_=iota16_i[:])
        ident_bf = small.tile([P, P], bf16, tag="ident")
        make_identity(nc, ident_bf[:])

        # st_si[k, 16q+l] = ((k mod 32) == 8*si + q): PE broadcast stationaries,
        # replicated every 32 partitions so lhsT can share rhs's base partition.
        iota_k = small.tile([P, 1], mybir.dt.int32, tag="iota_k")
        nc.gpsimd.iota(iota_k[:], pattern=[[0, 1]], base=0, channel_multiplier=1)
        iota_kf = small.tile([P, 1], f32, tag="iota_kf")
        nc.vector.tensor_copy(out=iota_kf[:], in_=iota_k[:])
        st_oct = []
        for o in range(16):
            st = small.tile([P, P], bf16, tag=f"st_oct{o}")
            for q in range(8):
                sl = st[:, 16 * q : 16 * q + 16]
                nc.vector.tensor_scalar(
                    out=sl,
                    in0=bass.AP(
                        tensor=iota_kf.tensor,
                        offset=iota_kf[:].offset,
                        ap=[iota_kf[:].ap[0], [0, 16]],
                    ),
                    scalar1=float(8 * o + q),
                    scalar2=None,
                    op0=mybir.AluOpType.is_equal,
                )
            st_oct.append(st)

        for b in range(BPC):
            # --- load ---
            p_t = big.tile([P, F], f32, tag="p")
            nc.sync.dma_start(out=p_t[:], in_=p_ext[b])
            ids_t = big.tile([P, F], mybir.dt.int32, tag="ids")
            nc.sync.dma_start(out=ids_t[:], in_=ids_ext[b])

            # contrib = (ids >= 0) * p, in bf16 (S only needs ~1e-3 rel)
            contrib_bf = big.tile([P, F], bf16, tag="contrib_bf")
            nc.vector.scalar_tensor_tensor(
                out=contrib_bf[:],
                in0=ids_t[:],
                scalar=0,
                in1=p_t[:],
                op0=mybir.AluOpType.is_ge,
                op1=mybir.AluOpType.mult,
            )

            # hi/lo via int ops; invalid ids (-1) produce hi >= 16 so every
            # hi-mask is false -> g=0 there, and contrib=0 keeps the hist exact
            t_i = big.tile([P, F], mybir.dt.int32, tag="idf")
            nc.vector.tensor_scalar(
                out=t_i[:],
                in0=ids_t[:],
                scalar1=4,
                scalar2=None,
                op0=mybir.AluOpType.logical_shift_right,
            )
            hi_bf = big.tile([P, F], bf16, tag="hi_bf")
            nc.scalar.copy(out=hi_bf[:], in_=t_i[:])
            t_i2 = big.tile([P, F], mybir.dt.int32, tag="idf")
            nc.vector.tensor_scalar(
                out=t_i2[:],
                in0=ids_t[:],
                scalar1=15,
                scalar2=None,
                op0=mybir.AluOpType.bitwise_and,
            )
            lo_bf = big.tile([P, F], bf16, tag="lo_bf")
            nc.scalar.copy(out=lo_bf[:], in_=t_i2[:])

            # --- histogram via PE diag-block matmuls ---
            # SP[16j+l, 16j+h] accumulates sum over pixels of V*U per 8-chunk set
            sp = pbig.tile([P, P], f32, tag="sp")
            n_chunks = F // P  # 64 transpose chunks
            first = True
            for c in range(n_chunks):
                cs = slice(c * P, (c + 1) * P)
                tr_ps = psum.tile([P, 3 * P], bf16, tag="tr")
                nc.tensor.transpose(
                    out=tr_ps[:, 0:P], in_=hi_bf[:, cs], identity=ident_bf[:]
                )
                nc.tensor.transpose(
                    out=tr_ps[:, P : 2 * P], in_=lo_bf[:, cs], identity=ident_bf[:]
                )
                nc.tensor.transpose(
                    out=tr_ps[:, 2 * P : 3 * P],
                    in_=contrib_bf[:, cs],
                    identity=ident_bf[:],
                )
                # ct must land in SBUF for the gpsimd multiply (no PSUM port)
                ctt = winp.tile([P, P], bf16, tag="trio")
                nc.scalar.copy(out=ctt[:], in_=tr_ps[:, 2 * P : 3 * P])

                # U[p, j, h] = (hi_t[p, j] == h); pV[p, j, l] = (lo_t==l)*ct
                # read hi_t/lo_t directly from PSUM (PSUM-source DVE ops run
                # at spec; SBUF-source pay the 2.3x cayman penalty)
                hit = tr_ps[:, 0:P]
                lot = tr_ps[:, P : 2 * P]
                ctt = ctt[:]
                # one fused op: [hi_t | lo_t] vs iota16 -> [u | v] (halves op overhead)
                uv_c = winp.tile([P, 2, P, 16], bf16, tag="u")
                nc.vector.tensor_tensor(
                    out=uv_c[:],
                    in0=bass.AP(
                        tensor=hit.tensor,
                        offset=hit.offset,
                        ap=[hit.ap[0], [P, 2], hit.ap[1], [0, 16]],
                    ),
                    in1=bass.AP(
                        tensor=iota16_bf.tensor,
                        offset=iota16_bf[:].offset,
                        ap=[iota16_bf[:].ap[0], [0, 2], [0, P], iota16_bf[:].ap[1]],
                    ),
                    op=mybir.AluOpType.is_equal,
                )
                u_c = uv_c[:, 0]
                v_c = uv_c[:, 1]
                nc.gpsimd.tensor_tensor(
                    out=v_c,
                    in0=v_c,
                    in1=bass.AP(
                        tensor=ctt.tensor,
                        offset=ctt.offset,
                        ap=[ctt.ap[0], ctt.ap[1], [0, 16]],
                    ),
                    op=mybir.AluOpType.mult,
                )
                for s in range(P // 8):
                    last = c == n_chunks - 1 and s == P // 8 - 1
                    nc.tensor.matmul(
                        out=sp[:],
                        lhsT=v_c[:, 8 * s : 8 * s + 8, :],
                        rhs=u_c[:, 8 * s : 8 * s + 8, :],
                        start=first,
                        stop=last,
                    )
                    first = False

            # --- extract S16T[l, h] = sum_q SP[16q+l, 16q+h] ---
            sp_sb = small.tile([P, P], f32, tag="sp_sb")
            nc.scalar.copy(out=sp_sb[:], in_=sp[:])
            sblk = small.tile([16, 16, 8], f32, tag="sblk")
            for q in range(8):
                nc.sync.dma_start(
                    out=sblk[:, :, q : q + 1],
                    in_=sp_sb[16 * q : 16 * q + 16, 16 * q : 16 * q + 16],
                )
            s16t = small.tile([16, 16], f32, tag="s16t")
            nc.vector.tensor_reduce(
                out=s16t[:],
                in_=sblk[:],
                axis=mybir.AxisListType.X,
                op=mybir.AluOpType.add,
            )
            # R = census/(S+eps), transposed layout [l, h]
            nc.vector.tensor_scalar(
                out=s16t[:],
                in0=s16t[:],
                scalar1=EPS,
                scalar2=None,
                op0=mybir.AluOpType.add,
            )
            nc.vector.reciprocal(out=s16t[:], in_=s16t[:])
            cen_t = small.tile([16, 16], f32, tag="cen_t")
            cb = cen_ext[b]
            nc.sync.dma_start(
                out=cen_t[:],
                in_=bass.AP(tensor=cb.tensor, offset=cb.offset, ap=[[1, 16], [16, 16]]),
            )
            rm_t = small.tile([16, 16], f32, tag="rm_t")
            nc.vector.tensor_tensor(
                out=rm_t[:], in0=cen_t[:], in1=s16t[:], op=mybir.AluOpType.mult
            )
            rm_tbf = small.tile([16, 16], bf16, tag="rm_tbf")
            nc.vector.tensor_copy(out=rm_tbf[:], in_=rm_t[:])
            # RmBD[16q+l, 16q+h] = Rm[h, l]
            for q in range(8):
                nc.sync.dma_start(
                    out=rm_bd[16 * q : 16 * q + 16, 16 * q : 16 * q + 16],
                    in_=rm_tbf[:],
                )

            # --- octet gather: g[p, f] = R[idc[p, f]] ---
            gat = big.tile([P, F], bf16, tag="ids")  # reuse ids slot
            strip = small.tile([8, F], bf16, tag="strip")
            for o in range(16):
                for w in range(F // WIN):
                    ws = slice(w * WIN, (w + 1) * WIN)
                    lbp = psum.tile([P, WIN], f32, tag="bc")
                    nc.tensor.matmul(
                        out=lbp[:],
                        lhsT=st_oct[o][:],
                        rhs=lo_bf[:, ws],
                        start=True,
                        stop=True,
                    )
                    vt = winsm.tile([P, WIN], bf16, tag="vt")
                    nc.vector.tensor_scalar(
                        out=vt[:],
                        in0=lbp[:],
                        scalar1=iota_l[:],
                        scalar2=None,
                        op0=mybir.AluOpType.is_equal,
                    )
                    wt = psum.tile([P, WIN], f32, tag="wt")
                    nc.tensor.matmul(
                        out=wt[:], lhsT=rm_bd[:], rhs=vt[:], start=True, stop=True
                    )
                    hbp = psum.tile([P, WIN], f32, tag="bc")
                    nc.tensor.matmul(
                        out=hbp[:],
                        lhsT=st_oct[o][:],
                        rhs=hi_bf[:, ws],
                        start=True,
                        stop=True,
                    )
                    wt_sb = winsm.tile([P, WIN], bf16, tag="hbw")
                    nc.scalar.copy(out=wt_sb[:], in_=wt[:])
                    prod = winsm.tile([P, WIN], bf16, tag="prod")
                    nc.vector.tensor_scalar(
                        out=prod[:],
                        in0=hbp[:],
                        scalar1=iota_l[:],
                        scalar2=None,
                        op0=mybir.AluOpType.is_equal,
                    )
                    nc.gpsimd.tensor_tensor(
                        out=prod[:], in0=prod[:], in1=wt_sb[:], op=mybir.AluOpType.mult
                    )
                    gps = pbig.tile([8, WIN], f32, tag="gps")
                    nc.tensor.matmul(
                        out=gps[:], lhsT=ones_bd[:], rhs=prod[:], start=True, stop=True
                    )
                    nc.scalar.copy(out=strip[:, ws], in_=gps[:])
                nc.sync.dma_start(out=gat[8 * o : 8 * o + 8, :], in_=strip[:])

            # --- out = contrib*g + (p - contrib), reusing p_t in place ---
            nc.gpsimd.tensor_tensor(
                out=gat[:], in0=contrib_bf[:], in1=gat[:], op=mybir.AluOpType.mult
            )
            nc.vector.tensor_tensor(
                out=p_t[:], in0=p_t[:], in1=contrib_bf[:], op=mybir.AluOpType.subtract
            )
            nc.vector.tensor_tensor(
                out=p_t[:], in0=p_t[:], in1=gat[:], op=mybir.AluOpType.add
            )
            nc.sync.dma_start(out=out_ext[b], in_=p_t[:])


def _build():
    nc = bacc.Bacc(
        "TRN2",
        target_bir_lowering=False,
        debug=False,
        enable_asserts=False,
        num_devices=NCORES,
    )
    f32 = mybir.dt.float32
    p_ext = nc.dram_tensor("p", [BPC, P, F], f32, kind="ExternalInput").ap()
    ids_ext = nc.dram_tensor(
        "ids", [BPC, P, F], mybir.dt.int32, kind="ExternalInput"
    ).ap()
    cen_ext = nc.dram_tensor("census", [BPC, A], f32, kind="ExternalInput").ap()
    out_ext = nc.dram_tensor("out", [BPC, P, F], f32, kind="ExternalOutput").ap()
    r_scratch = nc.dram_tensor("r_scratch", [1, A], f32).ap()
    with tile.TileContext(nc) as tc:
        _build_body(nc, tc, p_ext, ids_ext, cen_ext, out_ext, r_scratch)
    nc.compile()
    nc.m = get_hw_module(nc.m)
    return nc


def _run(P_raw, admin_ids, census_totals, trace=False):
    nc = _cache.get("nc")
    if nc is None:
        nc = _cache["nc"] = _build()
    in_maps = []
    for c in range(NCORES):
        sl = slice(c * BPC, (c + 1) * BPC)
        in_maps.append(
            {
                "p": np.ascontiguousarray(
                    np.asarray(P_raw[sl], dtype=np.float32).reshape(BPC, P, F)
                ),
                "ids": np.ascontiguousarray(
                    np.asarray(admin_ids[sl], dtype=np.int32).reshape(BPC, P, F)
                ),
                "census": np.ascontiguousarray(
                    np.asarray(census_totals[sl], dtype=np.float32)
                ),
            }
        )
    res = run_bass_kernel_spmd(nc, in_maps, list(range(NCORES)), trace=trace)
    out = np.concatenate(
        [res.results[c]["out"].reshape(BPC, 1, H, W) for c in range(NCORES)], axis=0
    ).astype(np.float32)
    return out, res


def kernel(P_raw, admin_ids, census_totals):
    out, _ = _run(P_raw, admin_ids, census_totals, trace=False)
    return out


# revision 37
# speedup vs baseline: 1.2259x; 1.0693x over previous
"""Census consistency layer (segment-sum + gather) on 8 Trainium2 cores.

Sharding: data-parallel over batch B=16 -> 8 cores x 2 batches each.
Per batch (1M pixels, 256 admin bins):
  S[a]   = sum of P_raw over pixels with admin_id == a   (masked, -1 invalid)
  R[a]   = census[a] / (S[a] + eps)
  out[n] = valid ? P_raw[n] * R[id[n]] : P_raw[n]
"""

import sys

sys.path.insert(0, "/opt/trn_rl_repo")

import numpy as np

import concourse.bacc as bacc
import concourse.tile as tile
from concourse import mybir
from concourse.bass_interp import get_hw_module
from concourse.bass_utils import run_bass_kernel_spmd

B, H, W, A = 16, 1024, 1024, 256
NCORES = 8
BPC = B // NCORES  # batches per core
P = 128
F = (H * W) // P  # free-dim elements per partition (8192)
EPS = 1e-6

_cache: dict = {}


def _build_body(nc, tc, p_ext, ids_ext, cen_ext, out_ext, r_scratch):
    import concourse.bass as bass
    from concourse.masks import make_identity

    f32 = mybir.dt.float32
    bf16 = mybir.dt.bfloat16
    WIN = 512
    with (
        tc.tile_pool(name="big", bufs=1) as big,
        tc.tile_pool(name="small", bufs=1) as small,
        tc.tile_pool(name="winp", bufs=3) as winp,
        tc.tile_pool(name="winsm", bufs=5) as winsm,
        tc.tile_pool(name="psum", bufs=2, space="PSUM") as psum,
        tc.tile_pool(name="pbig", bufs=1, space="PSUM") as pbig,
    ):
        ones = small.tile([P, 1], f32, tag="ones")
        nc.vector.memset(ones, 1.0)

        # iotaL[p] = p % 16 as f32 [128,1]
        iota_i = small.tile([P, 1], mybir.dt.int32, tag="iota_i")
        nc.gpsimd.iota(iota_i[:], pattern=[[0, 1]], base=0, channel_multiplier=1)
        nc.vector.tensor_scalar(
            out=iota_i[:],
            in0=iota_i[:],
            scalar1=15,
            scalar2=None,
            op0=mybir.AluOpType.bitwise_and,
        )
        iota_l = small.tile([P, 1], f32, tag="iota_l")
        nc.vector.tensor_copy(out=iota_l[:], in_=iota_i[:])

        # bd-structured PE constants: ones_bd[16q+l, q] = 1
        # (engine APs must start at aligned partitions, so build via
        #  full-partition compares against q = p >> 4)
        hiq_i = small.tile([P, 1], mybir.dt.int32, tag="hiq_i")
        nc.gpsimd.iota(hiq_i[:], pattern=[[0, 1]], base=0, channel_multiplier=1)
        nc.vector.tensor_scalar(
            out=hiq_i[:],
            in0=hiq_i[:],
            scalar1=4,
            scalar2=None,
            op0=mybir.AluOpType.logical_shift_right,
        )
        hiq_f = small.tile([P, 1], f32, tag="hiq_f")
        nc.vector.tensor_copy(out=hiq_f[:], in_=hiq_i[:])
        ones_bd = small.tile([P, 8], bf16, tag="ones_bd")
        for q in range(8):
            nc.vector.tensor_scalar(
                out=ones_bd[:, q : q + 1],
                in0=hiq_f[:],
                scalar1=float(q),
                scalar2=None,
                op0=mybir.AluOpType.is_equal,
            )
        rm_bd = small.tile([P, P], bf16, tag="rm_bd")
        nc.vector.memset(rm_bd[:], 0.0)

        iota16_i = small.tile([P, 16], mybir.dt.int32, tag="iota16_i")
        nc.gpsimd.iota(iota16_i[:], pattern=[[1, 16]], base=0, channel_multiplier=0)
        iota16_bf = small.tile([P, 16], bf16, tag="iota16_bf")
        nc.vector.tensor_copy(out=iota16_bf[:], in_=iota16_i[:])
        ident_bf = small.tile([P, P], bf16, tag="ident")
        make_identity(nc, ident_bf[:])

        # st_si[k, 16q+l] = ((k mod 32) == 8*si + q): PE broadcast stationaries,
        # replicated every 32 partitions so lhsT can share rhs's base partition.
        iota_k = small.tile([P, 1], mybir.dt.int32, tag="iota_k")
        nc.gpsimd.iota(iota_k[:], pattern=[[0, 1]], base=0, channel_multiplier=1)
        iota_kf = small.tile([P, 1], f32, tag="iota_kf")
        nc.vector.tensor_copy(out=iota_kf[:], in_=iota_k[:])
        st_oct = []
        for o in range(16):
            st = small.tile([P, P], bf16, tag=f"st_oct{o}")
            for q in range(8):
                sl = st[:, 16 * q : 16 * q + 16]
                nc.vector.tensor_scalar(
                    out=sl,
                    in0=bass.AP(
                        tensor=iota_kf.tensor,
                        offset=iota_kf[:].offset,
                        ap=[iota_kf[:].ap[0], [0, 16]],
                    ),
                    scalar1=float(8 * o + q),
                    scalar2=None,
                    op0=mybir.AluOpType.is_equal,
                )
            st_oct.append(st)

        for b in range(BPC):
            # --- load ---
            p_t = big.tile([P, F], f32, tag="p")
            nc.sync.dma_start(out=p_t[:], in_=p_ext[b])
            ids_t = big.tile([P, F], mybir.dt.int32, tag="ids")
            nc.sync.dma_start(out=ids_t[:], in_=ids_ext[b])

            # contrib = (ids >= 0) * p, in bf16 (S only needs ~1e-3 rel)
            contrib_bf = big.tile([P, F], bf16, tag="contrib_bf")
            nc.vector.scalar_tensor_tensor(
                out=contrib_bf[:],
                in0=ids_t[:],
                scalar=0,
                in1=p_t[:],
                op0=mybir.AluOpType.is_ge,
                op1=mybir.AluOpType.mult,
            )

            # hi/lo via int ops; invalid ids (-1) produce hi >= 16 so every
            # hi-mask is false -> g=0 there, and contrib=0 keeps the hist exact
            t_i = big.tile([P, F], mybir.dt.int32, tag="idf")
            nc.vector.tensor_scalar(
                out=t_i[:],
                in0=ids_t[:],
                scalar1=4,
                scalar2=None,
                op0=mybir.AluOpType.logical_shift_right,
            )
            hi_bf = big.tile([P, F], bf16, tag="hi_bf")
            nc.scalar.copy(out=hi_bf[:], in_=t_i[:])
            t_i2 = big.tile([P, F], mybir.dt.int32, tag="idf")
            nc.vector.tensor_scalar(
                out=t_i2[:],
                in0=ids_t[:],
                scalar1=15,
                scalar2=None,
                op0=mybir.AluOpType.bitwise_and,
            )
            lo_bf = big.tile([P, F], bf16, tag="lo_bf")
            nc.scalar.copy(out=lo_bf[:], in_=t_i2[:])

            # --- histogram via PE diag-block matmuls ---
            # SP[16j+l, 16j+h] accumulates sum over pixels of V*U per 8-chunk set
            sp = pbig.tile([P, P], f32, tag="sp")
            n_chunks = F // P  # 64 transpose chunks
            first = True
            for c in range(n_chunks):
                cs = slice(c * P, (c + 1) * P)
                tr_ps = psum.tile([P, 3 * P], bf16, tag="tr")
                nc.tensor.transpose(
                    out=tr_ps[:, 0:P], in_=hi_bf[:, cs], identity=ident_bf[:]
                )
                nc.tensor.transpose(
                    out=tr_ps[:, P : 2 * P], in_=lo_bf[:, cs], identity=ident_bf[:]
                )
                nc.tensor.transpose(
                    out=tr_ps[:, 2 * P : 3 * P],
                    in_=contrib_bf[:, cs],
                    identity=ident_bf[:],
                )
                # ct must land in SBUF for the gpsimd multiply (no PSUM port)
                ctt = winp.tile([P, P], bf16, tag="trio")
                nc.scalar.copy(out=ctt[:], in_=tr_ps[:, 2 * P : 3 * P])

                # U[p, j, h] = (hi_t[p, j] == h); pV[p, j, l] = (lo_t==l)*ct
                # read hi_t/lo_t directly from PSUM (PSUM-source DVE ops run
                # at spec; SBUF-source pay the 2.3x cayman penalty)
                hit = tr_ps[:, 0:P]
                lot = tr_ps[:, P : 2 * P]
                ctt = ctt[:]
                # one fused op: [hi_t | lo_t] vs iota16 -> [u | v] (halves op overhead)
                uv_c = winp.tile([P, 2, P, 16], bf16, tag="u")
                nc.vector.tensor_tensor(
                    out=uv_c[:],
                    in0=bass.AP(
                        tensor=hit.tensor,
                        offset=hit.offset,
                        ap=[hit.ap[0], [P, 2], hit.ap[1], [0, 16]],
                    ),
                    in1=bass.AP(
                        tensor=iota16_bf.tensor,
                        offset=iota16_bf[:].offset,
                        ap=[iota16_bf[:].ap[0], [0, 2], [0, P], iota16_bf[:].ap[1]],
                    ),
                    op=mybir.AluOpType.is_equal,
                )
                u_c = uv_c[:, 0]
                v_c = uv_c[:, 1]
                nc.gpsimd.tensor_tensor(
                    out=v_c,
                    in0=v_c,
                    in1=bass.AP(
                        tensor=ctt.tensor,
                        offset=ctt.offset,
                        ap=[ctt.ap[0], ctt.ap[1], [0, 16]],
                    ),
                    op=mybir.AluOpType.mult,
                )
                for s in range(P // 8):
                    last = c == n_chunks - 1 and s == P // 8 - 1
                    nc.tensor.matmul(
                        out=sp[:],
                        lhsT=v_c[:, 8 * s : 8 * s + 8, :],
                        rhs=u_c[:, 8 * s : 8 * s + 8, :],
                        start=first,
                        stop=last,
                    )
                    first = False

            # --- extract S16T[l, h] = sum_q SP[16q+l, 16q+h] ---
            sp_sb = small.tile([P, P], f32, tag="sp_sb")
            nc.scalar.copy(out=sp_sb[:], in_=sp[:])
            sblk = small.tile([16, 16, 8], f32, tag="sblk")
            for q in range(8):
                nc.sync.dma_start(
                    out=sblk[:, :, q : q + 1],
                    in_=sp_sb[16 * q : 16 * q + 16, 16 * q : 16 * q + 16],
                )
            s16t = small.tile([16, 16], f32, tag="s16t")
            nc.vector.tensor_reduce(
                out=s16t[:],
                in_=sblk[:],
                axis=mybir.AxisListType.X,
                op=mybir.AluOpType.add,
            )
            # R = census/(S+eps), transposed layout [l, h]
            nc.vector.tensor_scalar(
                out=s16t[:],
                in0=s16t[:],
                scalar1=EPS,
                scalar2=None,
                op0=mybir.AluOpType.add,
            )
            nc.vector.reciprocal(out=s16t[:], in_=s16t[:])
            cen_t = small.tile([16, 16], f32, tag="cen_t")
            cb = cen_ext[b]
            nc.sync.dma_start(
                out=cen_t[:],
                in_=bass.AP(tensor=cb.tensor, offset=cb.offset, ap=[[1, 16], [16, 16]]),
            )
            rm_t = small.tile([16, 16], f32, tag="rm_t")
            nc.vector.tensor_tensor(
                out=rm_t[:], in0=cen_t[:], in1=s16t[:], op=mybir.AluOpType.mult
            )
            rm_tbf = small.tile([16, 16], bf16, tag="rm_tbf")
            nc.vector.tensor_copy(out=rm_tbf[:], in_=rm_t[:])
            # RmBD[16q+l, 16q+h] = Rm[h, l]
            for q in range(8):
                nc.sync.dma_start(
                    out=rm_bd[16 * q : 16 * q + 16, 16 * q : 16 * q + 16],
                    in_=rm_tbf[:],
                )

            # --- octet gather: g[p, f] = R[idc[p, f]] ---
            gat = big.tile([P, F], bf16, tag="ids")  # reuse ids slot
            strip = small.tile([8, F], bf16, tag="strip")
            for o in range(16):
                for w in range(F // WIN):
                    ws = slice(w * WIN, (w + 1) * WIN)
                    lbp = psum.tile([P, WIN], f32, tag="bc")
                    nc.tensor.matmul(
                        out=lbp[:],
                        lhsT=st_oct[o][:],
                        rhs=lo_bf[:, ws],
                        start=True,
                        stop=True,
                    )
                    vt = winsm.tile([P, WIN], bf16, tag="vt")
                    nc.vector.tensor_scalar(
                        out=vt[:],
                        in0=lbp[:],
                        scalar1=iota_l[:],
                        scalar2=None,
                        op0=mybir.AluOpType.is_equal,
                    )
                    wt = psum.tile([P, WIN], f32, tag="wt")
                    nc.tensor.matmul(
                        out=wt[:], lhsT=rm_bd[:], rhs=vt[:], start=True, stop=True
                    )
                    hbp = psum.tile([P, WIN], f32, tag="bc")
                    nc.tensor.matmul(
                        out=hbp[:],
                        lhsT=st_oct[o][:],
                        rhs=hi_bf[:, ws],
                        start=True,
                        stop=True,
                    )
                    wt_sb = winsm.tile([P, WIN], bf16, tag="hbw")
                    nc.scalar.copy(out=wt_sb[:], in_=wt[:])
                    prod = winsm.tile([P, WIN], bf16, tag="prod")
                    nc.vector.tensor_scalar(
                        out=prod[:],
                        in0=hbp[:],
                        scalar1=iota_l[:],
                        scalar2=None,
                        op0=mybir.AluOpType.is_equal,
                    )
                    nc.gpsimd.tensor_tensor(
                        out=prod[:], in0=prod[:], in1=wt_sb[:], op=mybir.AluOpType.mult
                    )
                    gps = pbig.tile([8, WIN], f32, tag="gps")
                    nc.tensor.matmul(
                        out=gps[:], lhsT=ones_bd[:], rhs=prod[:], start=True, stop=True
                    )
                    nc.scalar.copy(out=strip[:, ws], in_=gps[:])
                nc.sync.dma_start(out=gat[8 * o : 8 * o + 8, :], in_=strip[:])

            # --- out = contrib*g + (p - contrib), reusing p_t in place ---
            nc.gpsimd.tensor_tensor(
                out=gat[:], in0=contrib_bf[:], in1=gat[:], op=mybir.AluOpType.mult
            )
            nc.vector.tensor_tensor(
                out=p_t[:], in0=p_t[:], in1=contrib_bf[:], op=mybir.AluOpType.subtract
            )
            nc.vector.tensor_tensor(
                out=p_t[:], in0=p_t[:], in1=gat[:], op=mybir.AluOpType.add
            )
            nc.sync.dma_start(out=out_ext[b], in_=p_t[:])


def _build():
    nc = bacc.Bacc(
        "TRN2",
        target_bir_lowering=False,
        debug=False,
        enable_asserts=False,
        num_devices=NCORES,
    )
    f32 = mybir.dt.float32
    p_ext = nc.dram_tensor("p", [BPC, P, F], f32, kind="ExternalInput").ap()
    ids_ext = nc.dram_tensor(
        "ids", [BPC, P, F], mybir.dt.int32, kind="ExternalInput"
    ).ap()
    cen_ext = nc.dram_tensor("census", [BPC, A], f32, kind="ExternalInput").ap()
    out_ext = nc.dram_tensor("out", [BPC, P, F], f32, kind="ExternalOutput").ap()
    r_scratch = nc.dram_tensor("r_scratch", [1, A], f32).ap()
    with tile.TileContext(nc) as tc:
        _build_body(nc, tc, p_ext, ids_ext, cen_ext, out_ext, r_scratch)
    nc.compile()
    nc.m = get_hw_module(nc.m)
    return nc


def _run(P_raw, admin_ids, census_totals, trace=False):
    nc = _cache.get("nc")
    if nc is None:
        nc = _cache["nc"] = _build()
    in_maps = []
    for c in range(NCORES):
        sl = slice(c * BPC, (c + 1) * BPC)
        in_maps.append(
            {
                "p": np.ascontiguousarray(
                    np.asarray(P_raw[sl], dtype=np.float32).reshape(BPC, P, F)
                ),
                "ids": np.ascontiguousarray(
                    np.asarray(admin_ids[sl], dtype=np.int32).reshape(BPC, P, F)
                ),
                "census": np.ascontiguousarray(
                    np.asarray(census_totals[sl], dtype=np.float32)
                ),
            }
        )
    res = run_bass_kernel_spmd(nc, in_maps, list(range(NCORES)), trace=trace)
    out = np.concatenate(
        [res.results[c]["out"].reshape(BPC, 1, H, W) for c in range(NCORES)], axis=0
    ).astype(np.float32)
    return out, res


def kernel(P_raw, admin_ids, census_totals):
    out, _ = _run(P_raw, admin_ids, census_totals, trace=False)
    return out
